# revision 5
# baseline (speedup 1.0000x reference)
"""Trainium2 Bass kernel for CudaTensorProduct (e3nn-style COO tensor product).

Computation: out[b, o] = sum_k cb[k] * in1[b, idx1[k]] * in2[b, idx2[k]]
  in1/in2: (16384, 32) f32, out: (16384, 1024) f32, nnz=4528.

Strategy (per core, pure data-parallel over batch, 2048 rows/core):
  - The COO table couples (i,j) input-pair columns to output columns. The
    bipartite graph decomposes into small connected components which we
    bin-pack into NG groups of (K<=128 ij-pairs, M<=128 out-cols).
  - Transpose inputs once: in12T (64, 2048) = [in1.T ; in2.T].
  - Per (group g, batch-chunk c of 512):
      R1 = E1g.T @ in12T_c   (PE, replicates in1 features to the group's K rows)
      R2 = E2g.T @ in12T_c   (PE, same for in2)
      U  = R1 * R2           (DVE elementwise -> the needed outer products)
      outT_gc = Wg.T @ U     (PE, the sparse-coefficient contraction)
    and DMA outT_gc to a (1024, 2048) transposed scratch output.
  - Host side un-transposes/un-permutes during the unshard (pure layout).

Matmuls run in float32r (TF32-like single-pass fp32) for 1 cyc/row.
"""

import os
import sys
import numpy as np

sys.path.insert(0, "/opt/trn_rl_repo")

import concourse.bass as bass
import concourse.mybir as mybir
import concourse.tile as tile
from concourse import bacc
from concourse.bass_utils import run_bass_kernel_spmd

N_CORES = 8
B = 16384
BC = B // N_CORES          # 2048 batch rows per core
D1 = 32
D2 = 32
DOUT = D1 * D2             # 1024
NG = 8                     # (K,M)<=128 groups
CHUNK = 512                # batch columns per matmul
NCHUNK = BC // CHUNK       # 4
F32 = mybir.dt.float32
F32R = mybir.dt.float32r


# ----------------------------------------------------------------------------
# Host-side table preprocessing
# ----------------------------------------------------------------------------

def _build_groups(idx1, idx2, out_idx, cb_vals):
    """Pack connected components of the (ij-col <-> out-row) graph into NG
    groups with K<=128 cols and M<=128 rows each.

    Returns (e12, w, rows_flat):
      e12: (64, NG*2*128) f32 — for group g, cols [2g*128,(2g+1)*128) hold
           E1g (rows 0:32 select i), cols [(2g+1)*128,(2g+2)*128) hold E2g
           (rows 32:64 select j).
      w:   (128, NG*128) f32 — w[:, g*128+m] holds the coefficients mapping
           group-g U rows to scratch out-row g*128+m.
      rows_flat: (NG*128,) int — scratch row r corresponds to real out col
           rows_flat[r] (-1 for padding, none expected here).
    """
    idx1 = np.asarray(idx1, np.int64)
    idx2 = np.asarray(idx2, np.int64)
    out_idx = np.asarray(out_idx, np.int64)
    cb = np.asarray(cb_vals, np.float64)
    col = idx1 * D2 + idx2

    parent = list(range(DOUT))

    def find(x):
        while parent[x] != x:
            parent[x] = parent[parent[x]]
            x = parent[x]
        return x

    col2row = {}
    for c, o in zip(col.tolist(), out_idx.tolist()):
        if c in col2row:
            ra, rb = find(col2row[c]), find(o)
            if ra != rb:
                parent[ra] = rb
        else:
            col2row[c] = o

    comp_rows, comp_cols = {}, {}
    for o in range(DOUT):
        comp_rows.setdefault(find(o), set()).add(o)
    for c, o in zip(col.tolist(), out_idx.tolist()):
        comp_cols.setdefault(find(o), set()).add(c)

    comps = [
        (sorted(comp_cols.get(k, ())), sorted(r)) for k, r in comp_rows.items()
    ]
    # drop out-rows with no terms (they are zero; none expected but be safe)
    comps = [(c, r) for c, r in comps if c]

    comps.sort(key=lambda cr: -len(cr[0]))
    bins = []
    for c, r in comps:
        for bn in bins:
            if bn["k"] + len(c) <= 128 and bn["m"] + len(r) <= 128:
                bn["cols"] += c
                bn["rows"] += r
                bn["k"] += len(c)
                bn["m"] += len(r)
                break
        else:
            bins.append({"cols": list(c), "rows": list(r), "k": len(c), "m": len(r)})
    assert len(bins) <= NG, f"packing produced {len(bins)} > {NG} groups"
    while len(bins) < NG:
        bins.append({"cols": [], "rows": [], "k": 0, "m": 0})

    # dense value map
    wmap = {}
    for c, o, v in zip(col.tolist(), out_idx.tolist(), cb.tolist()):
        wmap[(o, c)] = wmap.get((o, c), 0.0) + v

    e12 = np.zeros((64, NG * 2 * 128), np.float32)
    w = np.zeros((128, NG * 128), np.float32)
    rows_flat = np.full(NG * 128, -1, np.int64)
    for g, bn in enumerate(bins):
        cols, rows = bn["cols"], bn["rows"]
        colpos = {c: p for p, c in enumerate(cols)}
        for p, c in enumerate(cols):
            i, j = divmod(c, D2)
            e12[i, (2 * g) * 128 + p] = 1.0
            e12[32 + j, (2 * g + 1) * 128 + p] = 1.0
        for m, o in enumerate(rows):
            rows_flat[g * 128 + m] = o
        rowpos = {o: m for m, o in enumerate(rows)}
        for o in rows:
            for c in cols:
                v = wmap.get((o, c))
                if v is not None:
                    w[colpos[c], g * 128 + rowpos[o]] = np.float32(v)
    return e12, w, rows_flat


# ----------------------------------------------------------------------------
# Device program
# ----------------------------------------------------------------------------

def _build_bass():
    nc = bacc.Bacc("TRN2", target_bir_lowering=False)

    in12h = nc.dram_tensor("in12h", [BC, D1 + D2], F32, kind="ExternalInput")
    e12 = nc.dram_tensor("e12", [64, NG * 2 * 128], F32R, kind="ExternalInput")
    identw = nc.dram_tensor("identw", [128, 128], F32, kind="ExternalInput")
    wgt = nc.dram_tensor("wgt", [128, NG * 128], F32R, kind="ExternalInput")
    outT = nc.dram_tensor("outT", [DOUT, BC], F32, kind="ExternalOutput")

    NTILE = BC // 128  # 16 batch tiles for the input transpose

    with tile.TileContext(nc) as tc:
        with (
            tc.tile_pool(name="const", bufs=1) as const_pool,
            tc.tile_pool(name="inbuf", bufs=1) as in_pool,
            tc.tile_pool(name="r1sb", bufs=3) as r1_pool,
            tc.tile_pool(name="usb", bufs=3) as u_pool,
            tc.tile_pool(name="osb", bufs=4) as o_pool,
        ):
            e_sb = const_pool.tile([64, NG * 2 * 128], F32R)
            nc.sync.dma_start(out=e_sb[:], in_=e12.ap())
            w_sb = const_pool.tile([128, NG * 128], F32R)
            nc.sync.dma_start(out=w_sb[:], in_=wgt.ap())
            ident = const_pool.tile([128, 128], F32)
            nc.sync.dma_start(out=ident[:], in_=identw.ap())

            # interleaved input staging: in12[p, t, 0:32]=in1, [p, t, 32:64]=in2
            in12 = in_pool.tile([128, NTILE * 64], F32)
            in12_3d = in12[:].rearrange("p (t d) -> p t d", d=64)
            nc.sync.dma_start(
                out=in12_3d[:],
                in_=in12h.ap().rearrange("(t p) d -> p t d", p=128),
            )

            in12T = in_pool.tile([64, BC], F32R)

            # Phase 1: transpose inputs -> in12T (64, BC)
            with tc.tile_pool(name="ps_t", bufs=2, space="PSUM") as ps_t_pool:
                for tq in range(NTILE // 4):
                    ps = ps_t_pool.tile([64, 512], F32)
                    for ti in range(4):
                        t = tq * 4 + ti
                        nc.tensor.transpose(
                            ps[:, ti * 128 : (ti + 1) * 128],
                            in12_3d[:, t, :],
                            ident[:],
                        )
                    nc.scalar.copy(
                        out=in12T[:, tq * 512 : (tq + 1) * 512], in_=ps[:]
                    )

            # Phase 2: per (chunk, group) pipeline
            with (
                tc.tile_pool(name="ps_r1", bufs=2, space="PSUM") as ps_r1_pool,
                tc.tile_pool(name="ps_r2", bufs=2, space="PSUM") as ps_r2_pool,
                tc.tile_pool(name="ps_o", bufs=2, space="PSUM") as ps_o_pool,
            ):
                it = 0
                for c in range(NCHUNK):
                    rhs = in12T[:, c * CHUNK : (c + 1) * CHUNK]
                    for g in range(NG):
                        ps_r1 = ps_r1_pool.tile([128, CHUNK], F32)
                        nc.tensor.matmul(
                            ps_r1[:],
                            lhsT=e_sb[:, (2 * g) * 128 : (2 * g + 1) * 128],
                            rhs=rhs,
                            start=True,
                            stop=True,
                        )
                        ps_r2 = ps_r2_pool.tile([128, CHUNK], F32)
                        nc.tensor.matmul(
                            ps_r2[:],
                            lhsT=e_sb[:, (2 * g + 1) * 128 : (2 * g + 2) * 128],
                            rhs=rhs,
                            start=True,
                            stop=True,
                        )
                        r1sb = r1_pool.tile([128, CHUNK], F32)
                        nc.scalar.copy(out=r1sb[:], in_=ps_r1[:])
                        u = u_pool.tile([128, CHUNK], F32R)
                        nc.vector.tensor_mul(u[:], ps_r2[:], r1sb[:])
                        ps_o = ps_o_pool.tile([128, CHUNK], F32)
                        nc.tensor.matmul(
                            ps_o[:],
                            lhsT=w_sb[:, g * 128 : (g + 1) * 128],
                            rhs=u[:],
                            start=True,
                            stop=True,
                        )
                        osb = o_pool.tile([128, CHUNK], F32)
                        # split PSUM->SBUF output copies between DVE and ACT
                        if it % 5 < 2:
                            nc.vector.tensor_copy(osb[:], ps_o[:])
                        else:
                            nc.scalar.copy(out=osb[:], in_=ps_o[:])
                        nc.sync.dma_start(
                            out=outT.ap()[
                                g * 128 : (g + 1) * 128, c * CHUNK : (c + 1) * CHUNK
                            ],
                            in_=osb[:],
                        )
                        it += 1
    nc.compile()
    return nc


# ----------------------------------------------------------------------------
# Entry point
# ----------------------------------------------------------------------------

_CACHE = {}


def kernel(in1, in2, cb_vals, idx1, idx2, out_idx):
    in1 = np.ascontiguousarray(np.asarray(in1, np.float32))
    in2 = np.ascontiguousarray(np.asarray(in2, np.float32))

    key = (
        np.asarray(idx1).tobytes(),
        np.asarray(idx2).tobytes(),
        np.asarray(out_idx).tobytes(),
        np.asarray(cb_vals).tobytes(),
    )
    kh = hash(key)
    if kh not in _CACHE:
        e12, w, rows_flat = _build_groups(idx1, idx2, out_idx, cb_vals)
        nc = _build_bass()
        _CACHE[kh] = (nc, e12, w, rows_flat)
    nc, e12, w, rows_flat = _CACHE[kh]

    ident = np.eye(128, dtype=np.float32)
    in12h = np.concatenate([in1, in2], axis=1)  # (B, 64)
    in_maps = []
    for core in range(N_CORES):
        sl = slice(core * BC, (core + 1) * BC)
        in_maps.append(
            {
                "in12h": np.ascontiguousarray(in12h[sl]),
                "e12": e12,
                "wgt": w,
                "identw": ident,
            }
        )

    trace = bool(int(os.environ.get("KERNEL_TRACE", "0")))
    res = run_bass_kernel_spmd(
        nc, in_maps, core_ids=list(range(N_CORES)), trace=trace
    )
    kernel.last_results = res

    out = np.empty((B, DOUT), np.float32)
    valid = rows_flat >= 0
    cols = rows_flat[valid]
    for core in range(N_CORES):
        shard = res.results[core]["outT"]  # (DOUT, BC) scratch layout
        blk = out[core * BC : (core + 1) * BC]
        blk[:, cols] = shard[valid].T
        if not valid.all():
            blk[:, ~np.isin(np.arange(DOUT), cols)] = 0.0
    return out
